# revision 34
# baseline (speedup 1.0000x reference)
"""Trainium2 Bass kernel for nn_Attention_55233279426826 (block-causal attention).

Reference computation (per batch b):
    xn = LayerNorm(x[b]) * gamma + beta
    q,k,v = split(xn @ w_qkv), 12 heads x 64
    attn  = softmax(block-causal-masked(q k^T / 8))
    out[b] = (attn v) @ w_out + b_out

Sharding (8 cores): batch (2) x head-group (4, 3 heads each).  Each core gets
its batch's x, the w_qkv columns and w_out rows of its 3 heads, and produces a
partial [2048, 768] output.  Host sums the 4 head-group partials per batch and
adds b_out.

Per-core device program (all matmuls in float32r = full-rate fp32):
  1. LayerNorm stats in [token, dim] layout (bn_stats/bn_aggr), apply
     (x - mu) * rstd.  gamma is folded into w_qkv on device; beta becomes a
     per-channel bias beta @ w_qkv added at the QKV psum eviction.
  2. PE-transpose xn -> xnT [768, 2048] (dim on partitions).
  3. qkvT [576, 2048] = w_qkv^T @ xnT.  Column order of the host-permuted
     w_qkv places each head's qT/kT at equal partition offsets so the score
     matmuls have matching operand base partitions.
  4. v is re-transposed to natural [keys, 64] layout, augmented with a ones
     column (index 64) so the attention A@V matmul also produces softmax
     denominators in psum row 64.
  5. Scores are computed transposed S_T[j, q] per 128-key block J and 512-query
     chunk c, exp'ed on the scalar engine (no max subtraction: |scores| <= ~8
     so exp is safe in fp32), masked corners zeroed, accumulated A@V into psum.
     The denominator row is reciprocal'ed, broadcast across 64 partitions via a
     K=1 matmul, and divided out during the psum->SBUF eviction multiply.
  6. out = O_cat^T @ w_out per 128-token tile, streamed to DRAM.
"""

import contextlib
import ctypes
import os
import sys
import types

import numpy as np

B = 2
T = 2048
D = 768
NPATCH = 64
HEADS = 12
DH = 64
NH = 3          # heads per core
CH = 3 * NH * DH  # 576 qkv channels per core
LN_EPS = 1e-5
NCORES = 8

_CACHE = {}


def _install_axon_hooks_shim():
    """This image's antenv lacks axon_hooks; synthesize it so that
    run_bass_kernel_spmd(trace=True) finds the NTFF profile hook instead of
    crashing on import.  Safe no-op if profiling symbols are unavailable."""
    if "antenv.axon_hooks" in sys.modules:
        return
    mod = types.ModuleType("antenv.axon_hooks")
    _hook = [None]
    mod.set_axon_ntff_profile_hook = lambda h: _hook.__setitem__(0, h)
    mod.get_axon_ntff_profile_hook = lambda: _hook[0]
    sys.modules["antenv.axon_hooks"] = mod
    try:
        lib = ctypes.CDLL("/opt/axon/libaxon_pjrt.so")
        if not hasattr(lib, "axon_start_nrt_profile"):
            return
        lib.axon_start_nrt_profile.argtypes = [
            ctypes.POINTER(ctypes.c_int64),
            ctypes.c_size_t,
        ]
        lib.axon_start_nrt_profile.restype = ctypes.c_int64
        lib.axon_stop_nrt_profile.argtypes = [ctypes.c_char_p]
        lib.axon_stop_nrt_profile.restype = ctypes.c_int64

        @contextlib.contextmanager
        def _hook_cm(output_dir, device_ids):
            import jax

            jax.devices()
            if device_ids:
                ids = (ctypes.c_int64 * len(device_ids))(*device_ids)
                rc = lib.axon_start_nrt_profile(ids, len(device_ids))
            else:
                rc = lib.axon_start_nrt_profile(None, 0)
            if rc != 0:
                raise RuntimeError(f"axon_start_nrt_profile rc={rc}")
            try:
                yield
            finally:
                n = lib.axon_stop_nrt_profile(str(output_dir).encode())
                print(f"profile: {n} file(s) -> {output_dir}", file=sys.stderr)

        mod.set_axon_ntff_profile_hook(_hook_cm)
    except OSError:
        pass


def _install_drain_split():
    """The walrus build in this container accepts only ONE sync wait per
    CTRL(drain) instruction; Tile's tail drain carries several.  Split the
    waits across a chain of drains."""
    import bass_rust
    import concourse.tile as tile
    from concourse.vector_clock import ScopedClock

    if getattr(tile.TileContext, "_drain_split_installed", False):
        return

    def _drain_and_barrier(self, tick_clock, wait_clock):
        nc = self.nc
        drain_inst = nc.sync.drain()
        wait_clock.add_sem_waits(
            drain_inst.ins, ScopedClock({None: tick_clock.global_clock})
        )
        si = drain_inst.ins.sync_info
        if si is not None:
            waits = list(si.on_wait)
            if len(waits) > 1:
                si.on_wait = waits[:1]
                for w in waits[1:]:
                    extra = nc.sync.drain()
                    extra.ins.sync_info = bass_rust.SyncInfo(
                        on_wait=[w], on_update=[]
                    )
        nc.all_engine_barrier()
        popped = nc._tile_sem_poison_stack.pop()
        assert popped is self._sem_poison
        nc.clear_and_free_semaphores(list(self.sems.allocated().values()))
        nc.all_engine_barrier()

    tile.TileContext._drain_and_barrier = _drain_and_barrier

    # Generic pass: walrus here allows 1 sync wait per instruction; move
    # extra waits onto nofuse NOPs inserted just before, on the same engine.
    from concourse import mybir

    orig_lower = tile.TileContext._lower_ordered_insts

    def _lower_split(self, ordered):
        for insts in ordered.values():
            new = []
            for inst in insts:
                si = getattr(inst, "sync_info", None)
                eng = getattr(inst, "engine", None)
                if si is not None and eng is not None:
                    waits = list(si.on_wait)
                    if len(waits) > 1:
                        movable = [w for w in waits
                                   if getattr(w, "sync_type", "") == "semaphore"]
                        keep = [w for w in waits if w not in movable]
                        if not keep:
                            keep = [movable.pop()]
                        for k, w in enumerate(movable):
                            nop = mybir.InstNoOp(
                                name=f"{inst.name}-wsplit{k}",
                                sync_info=mybir.SyncInfo(
                                    on_wait=[w], on_update=[]
                                ),
                                bass_nofuse=True,
                                engine=eng,
                            )
                            new.append(nop)
                        inst.sync_info = mybir.SyncInfo(
                            on_wait=keep, on_update=list(si.on_update)
                        )
                new.append(inst)
            insts[:] = new
        return orig_lower(self, ordered)

    tile.TileContext._lower_ordered_insts = _lower_split
    tile.TileContext._drain_split_installed = True


# qkvT row layout: which [128/64, 2048] tile and partition offset holds each
# head's 64-row qT/kT/vT strip.  q and k of the same head share a partition
# offset (matmul operands must have equal base partitions).
Q_LOC = [(0, 0), (0, 64), (2, 0)]
K_LOC = [(1, 0), (1, 64), (3, 0)]
V_LOC = [(2, 64), (3, 64), (4, 0)]
# host column order of the permuted per-core w_qkv (64-col segments)
# tile0 = [q0; q1], tile1 = [k0; k1], tile2 = [q2; v0], tile3 = [k2; v1],
# tile4 = [v2]
SEG_ORDER = [("q", 0), ("q", 1), ("k", 0), ("k", 1), ("q", 2), ("v", 0),
             ("k", 2), ("v", 1), ("v", 2)]

C_CHUNKS = [(0, 128), (128, 128), (256, 128), (384, 128), (512, 64)]


def build_nc():
    import concourse.bass as bass
    import concourse.tile as tile
    from concourse import mybir
    from concourse.masks import make_identity

    _install_drain_split()

    f32 = mybir.dt.float32
    f32r = mybir.dt.float32r
    bf16 = mybir.dt.bfloat16
    AF = mybir.ActivationFunctionType
    Alu = mybir.AluOpType

    def as32(ap):
        return ap.bitcast(f32)

    nc = bass.Bass()
    x_d = nc.dram_tensor("x", [T, D], f32, kind="ExternalInput")
    wqkv_d = nc.dram_tensor("wqkv", [D, CH], f32, kind="ExternalInput")
    wout_d = nc.dram_tensor("wout", [NH * DH, D], f32, kind="ExternalInput")
    gamma_d = nc.dram_tensor("gamma", [D], f32, kind="ExternalInput")
    beta_d = nc.dram_tensor("beta", [D], f32, kind="ExternalInput")
    out_d = nc.dram_tensor("out", [T, D], f32, kind="ExternalOutput")

    with contextlib.ExitStack() as ctx:
        ctx.enter_context(
            nc.allow_low_precision(reason="bf16 PE inputs are intentional")
        )
        tc = ctx.enter_context(tile.TileContext(nc))
        consts = ctx.enter_context(tc.tile_pool(name="consts", bufs=1))
        wpool = ctx.enter_context(tc.tile_pool(name="w", bufs=1))
        qkvT_pool = ctx.enter_context(tc.tile_pool(name="qkvT", bufs=1))
        vaug_pool = ctx.enter_context(tc.tile_pool(name="vaug", bufs=1))
        ocat_pool = ctx.enter_context(tc.tile_pool(name="ocat", bufs=1))
        io_pool = ctx.enter_context(tc.tile_pool(name="io", bufs=3))
        stats = ctx.enter_context(tc.tile_pool(name="stats", bufs=4))

        identity = consts.tile([128, 128], f32, tag="id")
        make_identity(nc, identity)
        id_bf = consts.tile([128, 128], bf16, tag="idbf")
        nc.vector.tensor_copy(id_bf, identity)
        eps_t = consts.tile([128, 1], f32, tag="eps")
        nc.vector.memset(eps_t, LN_EPS)
        ones_t = consts.tile([128, DH], f32r, tag="ones")
        nc.vector.memset(as32(ones_t), 1.0)
        gamma_t = consts.tile([128, 6], f32, tag="gam")
        nc.sync.dma_start(gamma_t, gamma_d[:].rearrange("(a p) -> p a", p=128))
        beta_t = consts.tile([128, 6], f32, tag="bet")
        nc.sync.dma_start(beta_t, beta_d[:].rearrange("(a p) -> p a", p=128))

        w_sb = []
        wout_sb = []
        bw_sb = []
        with (
            tc.tile_pool(name="wraw", bufs=1) as wraw,
            tc.tile_pool(name="bw_ps", bufs=2, space="PSUM") as bw_ps,
        ):
            w_raw = []
            for j in range(6):
                wt = wraw.tile([128, CH], f32, tag=f"wr{j}", name=f"wr{j}")
                nc.sync.dma_start(wt, wqkv_d[128 * j : 128 * (j + 1), :])
                w_raw.append(wt)
            wout_raw = []
            for h in range(NH):
                wo = wraw.tile([64, D], f32, tag=f"wor{h}", name=f"wor{h}")
                nc.sync.dma_start(wo, wout_d[64 * h : 64 * (h + 1), :])
                wout_raw.append(wo)

            # beta @ w_qkv (raw weights), one [csz,1] psum per c-chunk
            for ci, (clo, csz) in enumerate(C_CHUNKS):
                ps = bw_ps.tile([128, 1], f32, tag="bw")
                for j in range(6):
                    nc.tensor.matmul(
                        ps[:csz, :],
                        w_raw[j][:, clo : clo + csz],
                        beta_t[:, j : j + 1],
                        start=(j == 0),
                        stop=(j == 5),
                    )
                bw = consts.tile([128, 1], f32, tag=f"bw{ci}", name=f"bw{ci}")
                nc.scalar.copy(bw[:csz, :], ps[:csz, :])
                bw_sb.append(bw)
            # fold gamma; bf16 output for the PE
            for j in range(6):
                wf = wpool.tile([128, CH], bf16, tag=f"w{j}", name=f"w{j}")
                nc.vector.tensor_scalar_mul(
                    wf[:], in0=w_raw[j][:], scalar1=gamma_t[:, j : j + 1]
                )
                w_sb.append(wf)
            for h in range(NH):
                wof = wpool.tile([64, D], bf16, tag=f"wo{h}", name=f"wo{h}")
                nc.vector.tensor_copy(wof[:], wout_raw[h][:])
                wout_sb.append(wof)

        qkvT = []
        for ci, (clo, csz) in enumerate(C_CHUNKS):
            qkvT.append(qkvT_pool.tile([csz, T], bf16, tag=f"qkvT{ci}", name=f"qkvT{ci}"))
        vaug = [vaug_pool.tile([128, 16, DH + 1], bf16, tag=f"va{h}", name=f"va{h}")
                for h in range(NH)]
        ocat = [ocat_pool.tile([64, T], bf16, tag=f"oc{h}", name=f"oc{h}") for h in range(NH)]
        for h in range(NH):
            nc.vector.memset(vaug[h][:, :, DH : DH + 1].bitcast(bf16), 1.0)

        with (
            tc.tile_pool(name="xn", bufs=2) as xn_pool,
            tc.tile_pool(name="xnT", bufs=1) as xnT_pool,
            tc.tile_pool(name="scr", bufs=2) as scr_pool,
            tc.tile_pool(name="xp_ps", bufs=2, space="PSUM") as xp_ps,
            tc.tile_pool(name="qkv_ps", bufs=1, space="PSUM") as qkv_ps,
        ):
            # ---- LayerNorm (stats on ACT via accum_out) + PE transpose.
            # Full xnT [768, 2048] in bf16; sub-tile deps let QKV start on a
            # 512-column chunk as soon as its transposes land.
            xnT = [xnT_pool.tile([128, T], bf16, tag=f"xnT{j}", name=f"xnT{j}")
                   for j in range(6)]
            for grp in range(4):
                xts = []
                for u in range(4):
                    i = 4 * grp + u
                    xt = io_pool.tile([128, D], f32, tag="xin", name="xin")
                    nc.sync.dma_start(xt, x_d[128 * i : 128 * (i + 1), :])
                    st = stats.tile([128, 3, 6], f32, tag="bnst", name="bnst")
                    for s in range(3):
                        nc.vector.bn_stats(
                            st[:, s, :], xt[:, 256 * s : 256 * (s + 1)]
                        )
                    mv = stats.tile([128, 2], f32, tag="mv", name="mv")
                    nc.vector.bn_aggr(mv, st)
                    rstd = stats.tile([128, 1], f32, tag="rstd", name="rstd")
                    nc.scalar.activation(rstd, mv[:, 1:2], AF.Sqrt, bias=eps_t)
                    nc.vector.reciprocal(rstd, rstd)
                    xn_t = xn_pool.tile([128, D], bf16, tag=f"xn{u}", name=f"xn{u}")
                    nc.vector.tensor_scalar(
                        out=xn_t,
                        in0=xt,
                        scalar1=mv[:, 0:1],
                        scalar2=rstd,
                        op0=Alu.subtract,
                        op1=Alu.mult,
                    )
                    xts.append(xn_t)
                for j in range(6):
                    ps = xp_ps.tile([128, 512], bf16, tag="xp", name="xp")
                    for u in range(4):
                        nc.tensor.transpose(
                            ps[:, 128 * u : 128 * (u + 1)],
                            xts[u][:, 128 * j : 128 * (j + 1)],
                            id_bf,
                        )
                    nc.vector.tensor_copy(
                        xnT[j][:, 512 * grp : 512 * (grp + 1)], ps
                    )
                # QKV for this token chunk (keeps the PE dense while the
                # next group's LayerNorm runs on ACT/DVE)
                for ci, (clo, csz) in enumerate(C_CHUNKS):
                    pq = qkv_ps.tile([128, 512], f32, tag=f"qk{ci % 2}",
                                     name=f"qk{ci % 2}")
                    for j in range(6):
                        nc.tensor.matmul(
                            pq[:csz, :],
                            w_sb[j][:, clo : clo + csz],
                            xnT[j][:, 512 * grp : 512 * (grp + 1)],
                            start=(j == 0),
                            stop=(j == 5),
                        )
                    nc.vector.tensor_scalar_add(
                        qkvT[ci][:csz, 512 * grp : 512 * (grp + 1)],
                        in0=pq[:csz, :],
                        scalar1=bw_sb[ci][:csz, :],
                    )
                # v natural rows for this group's 4 key blocks
                for h in range(NH):
                    tI, ro = V_LOC[h]
                    idsl = id_bf[ro : ro + 64, ro : ro + 64]
                    ps = xp_ps.tile([128, 512], bf16, tag="xp", name="xp")
                    for u in range(4):
                        J = 4 * grp + u
                        nc.tensor.transpose(
                            ps[:, 64 * u : 64 * (u + 1)],
                            qkvT[tI][ro : ro + 64, 128 * J : 128 * (J + 1)],
                            idsl,
                        )
                    nc.vector.tensor_copy(
                        vaug[h][:, 4 * grp : 4 * (grp + 1), 0:DH],
                        ps[:, 0:256].rearrange("p (u d) -> p u d", u=4),
                    )

            # (QKV and v-transposes are fused into the per-group loop above)

        # ---- attention + inline out-projection.
        # c outer with a one-J lookahead; within (c, J) the three heads'
        # S -> exp -> A@V chains interleave so no engine starves.  After a
        # chunk finalizes, its four token tiles are projected immediately.
        with (
            tc.tile_pool(name="st_ps", bufs=3, space="PSUM") as st_ps,
            tc.tile_pool(name="ot_ps", bufs=1, space="PSUM") as ot_ps,
            tc.tile_pool(name="bc_ps", bufs=1, space="PSUM") as bc_ps,
            tc.tile_pool(name="op_ps", bufs=1, space="PSUM") as op_ps,
            tc.tile_pool(name="pt", bufs=3) as pt_pool,
            tc.tile_pool(name="rec", bufs=2) as rec_pool,
        ):
            for c in range(4):
                otp = [ot_ps.tile([DH + 1, 512], f32, tag=f"ot{h}", name=f"ot{h}")
                       for h in range(NH)]
                nJ = 4 * c + 4
                pending = []

                def emit_av(Jp, s0p, np_, ptsp):
                    for h in range(NH):
                        nc.tensor.matmul(
                            otp[h][:, s0p:512],
                            vaug[h][:, Jp, :],
                            ptsp[h][:, :np_],
                            start=(Jp == 0),
                            stop=(Jp == nJ - 1),
                        )

                for J in range(nJ):
                    s0 = max(0, 128 * J - 512 * c)
                    n = 512 - s0
                    q0 = 512 * c + s0
                    pts = []
                    for h in range(NH):
                        qt, qo = Q_LOC[h]
                        kt, ko = K_LOC[h]
                        stp = st_ps.tile([128, 512], f32, tag="st", name="st")
                        nc.tensor.matmul(
                            stp[:, :n],
                            qkvT[kt][ko : ko + 64, 128 * J : 128 * (J + 1)],
                            qkvT[qt][qo : qo + 64, q0 : q0 + n],
                            start=True,
                            stop=True,
                        )
                        pt = pt_pool.tile([128, 512], bf16, tag=f"pt{h}", name=f"pt{h}")
                        nc.scalar.activation(
                            pt[:, :n], stp[:, :n], AF.Exp,
                            scale=float(DH) ** -0.5,
                        )
                        if J >= 4 * c:
                            nc.vector.memset(pt[64:128, 0:64].bitcast(bf16), 0.0)
                        pts.append(pt)
                    pending.append((J, s0, n, pts))
                    if len(pending) > 1:
                        emit_av(*pending.pop(0))
                while pending:
                    emit_av(*pending.pop(0))

                for h in range(NH):
                    # 1/den via exp(-log(den)) on ACT (DVE reciprocal is ~6
                    # passes, 3.3us per call), broadcast the reciprocal row
                    ld = rec_pool.tile([128, 512], f32, tag="ld", name="ld")
                    nc.scalar.activation(
                        ld[64:65, :], otp[h][64:65, :], AF.Ln
                    )
                    den = rec_pool.tile([128, 512], f32r, tag="den", name="den")
                    nc.scalar.activation(
                        den[64:65, :], ld[64:65, :], AF.Exp, scale=-1.0
                    )
                    bcp = bc_ps.tile([64, 512], f32, tag="bc", name="bc")
                    nc.tensor.matmul(
                        bcp,
                        ones_t[64:65, 0:DH],
                        den[64:65, :],
                        start=True,
                        stop=True,
                    )
                    recs = rec_pool.tile([64, 512], f32, tag="recs", name="recs")
                    nc.vector.tensor_copy(recs, bcp)
                    nc.vector.tensor_mul(
                        ocat[h][:, 512 * c : 512 * (c + 1)],
                        otp[h][0:DH, :],
                        recs[:, :],
                    )
                # out projection for this chunk's four token tiles
                for t in range(4 * c, 4 * c + 4):
                    ot_sb = io_pool.tile([128, D], f32, tag="osb", name="osb")
                    for eh in range(2):
                        opp = op_ps.tile([128, 384], f32, tag="op", name="op")
                        for hh in range(NH):
                            nc.tensor.matmul(
                                opp,
                                ocat[hh][:, 128 * t : 128 * (t + 1)],
                                wout_sb[hh][:, 384 * eh : 384 * (eh + 1)],
                                start=(hh == 0),
                                stop=(hh == NH - 1),
                            )
                        nc.vector.tensor_copy(
                            ot_sb[:, 384 * eh : 384 * (eh + 1)], opp
                        )
                    nc.sync.dma_start(out_d[128 * t : 128 * (t + 1), :], ot_sb)

    return nc





def shard_inputs(x, gamma, beta, w_qkv, w_out, b_out):
    """Full inputs -> list of 8 per-core input dicts."""
    x = np.ascontiguousarray(np.asarray(x, dtype=np.float32))
    gamma = np.asarray(gamma, dtype=np.float32)
    beta = np.asarray(beta, dtype=np.float32)
    w_qkv = np.asarray(w_qkv, dtype=np.float32)
    w_out = np.asarray(w_out, dtype=np.float32)
    in_maps = []
    for g in range(NCORES):
        b = g // 4
        hg = g % 4
        heads = [3 * hg + h for h in range(NH)]
        segs = []
        for kind, h in SEG_ORDER:
            hh = heads[h]
            base = {"q": 0, "k": D, "v": 2 * D}[kind]
            segs.append(w_qkv[:, base + 64 * hh : base + 64 * (hh + 1)])
        wqkv_g = np.ascontiguousarray(np.concatenate(segs, axis=1))
        wout_g = np.ascontiguousarray(
            w_out[64 * heads[0] : 64 * (heads[-1] + 1), :]
        )
        in_maps.append(
            {
                "x": x[b],
                "wqkv": wqkv_g,
                "wout": wout_g,
                "gamma": gamma,
                "beta": beta,
            }
        )
    return in_maps


def kernel(x, gamma, beta, w_qkv, w_out, b_out):
    _install_axon_hooks_shim()
    from concourse import bass_utils

    if "nc" not in _CACHE:
        _CACHE["nc"] = build_nc()
    nc = _CACHE["nc"]

    in_maps = shard_inputs(x, gamma, beta, w_qkv, w_out, b_out)
    trace = bool(int(os.environ.get("KERNEL_TRACE", "0")))
    kwargs = {}
    if trace:
        kwargs["trace"] = True
        tmpdir = os.environ.get("KERNEL_TRACE_DIR")
        if tmpdir:
            kwargs["tmpdir"] = tmpdir
        # artifact upload needs external storage; keep the trace local
        bass_utils.upload_artifacts = lambda d: d
    res = bass_utils.run_bass_kernel_spmd(
        nc, in_maps, list(range(NCORES)), **kwargs
    )
    _CACHE["last_exec_time_ns"] = res.exec_time_ns

    b_out = np.asarray(b_out, dtype=np.float32)
    out = np.empty((B, T, D), dtype=np.float32)
    for b in range(B):
        acc = res.results[4 * b]["out"].astype(np.float32)
        for hg in range(1, 4):
            acc = acc + res.results[4 * b + hg]["out"]
        out[b] = acc + b_out[None, :]
    return out
